# revision 12
# baseline (speedup 1.0000x reference)
"""Expert-parallel MoE (BailingMoeV25-style) kernel for 8 trn2 NeuronCores.

Strategy (v2):
  - Host computes routing (exact numpy replica of the reference _route).
    The routing is heavily skewed (few experts receive nearly all tokens),
    so each core loads TWO full expert weight sets ("regions" A and B, bf16)
    and processes two token batches ("slots") of template sizes (CA, CB).
    The template (CA, CB) and the (expert, token-chunk) -> (core, region)
    assignment are chosen at runtime by a small search; the program is
    identical on all cores (SPMD), only the data differs.
  - Matmuls run in "token-free" orientation: out[features, tokens] with the
    weight matrices as natural-layout stationary operands (lhsT) and x^T as
    the moving operand. This needs no on-chip transposes, and the cost
    scales with the token count.  All matmul inputs are bf16 (f32 PSUM
    accumulate); y partials are returned in bf16.
  - Combine weights (2.5 * top-k weight, 1.0 for the shared expert) are
    applied on the host during the scatter-add combine, so slots need no
    on-chip scaling.
  - The shared expert is just another job (expert id -1) with all T tokens.
  - Jobs that cannot be packed into the 8x(A,B) windows (a couple of
    near-empty experts) are computed on the host in f32 (<=0.5% of tokens).
"""
import math
import sys

import numpy as np

if '/opt/trn_rl_repo' not in sys.path:
    sys.path.insert(0, '/opt/trn_rl_repo')

P = 128
T, H, E, I = 1024, 2048, 32, 512
KC = H // P          # 16 contraction chunks of the hidden dim
IC = I // P          # 4 chunks of the intermediate dim
HC = H // P          # 16 output chunks of the hidden dim
TOP_K, N_GROUP, TOPK_GROUP = 4, 4, 2
ROUTED_SCALING = 2.5
N_CORES = 8

# cost-model constants used only for template scoring (ns)
_WT_NS = 17476.0       # one bf16 expert weight set (6.29 MB) @ 360 GB/s
_TOK_DMA_NS = 22.8     # xt + y bytes per token (8 KB bf16) @ 360 GB/s
_TOK_PE_NS = 80.0      # 192 PE rows per token @ 2.4 GHz


def route_np(x, gw, eb):
    """Exact numpy replica of reference._route (fp32)."""
    x = np.asarray(x, np.float32)
    gw = np.asarray(gw, np.float32)
    eb = np.asarray(eb, np.float32)
    logits = x @ gw.T
    scores = np.float32(1.0) / (np.float32(1.0) + np.exp(-logits, dtype=np.float32))
    sc = scores + eb[None, :]
    t, e = scores.shape
    g = e // N_GROUP
    grp = sc.reshape(t, N_GROUP, g)
    top2 = np.sort(grp, axis=-1)[:, :, -2:]
    group_scores = top2.sum(-1)
    grp_idx = np.argsort(-group_scores, kind='stable', axis=-1)[:, :TOPK_GROUP]
    gmask = np.zeros((t, N_GROUP), bool)
    gmask[np.arange(t)[:, None], grp_idx] = True
    emask = np.repeat(gmask, g, axis=1)
    masked = np.where(emask, sc, -np.inf)
    topk_ids = np.argsort(-masked, kind='stable', axis=-1)[:, :TOP_K]
    w = np.take_along_axis(scores, topk_ids, axis=1)
    w = w / w.sum(-1, keepdims=True)
    Wm = np.zeros((t, e), np.float32)
    np.put_along_axis(Wm, topk_ids, w.astype(np.float32), axis=1)
    return Wm


def _try_pack(sizes, CA, CB):
    """Can jobs of the given token counts be split into at most 8 A-pieces
    (each <= CA) and 8 B-pieces (each <= CB)?  Returns per-job A-window
    counts k_j, or None."""
    n = len(sizes)

    def b_windows(ks):
        tot = 0
        for t_j, k in zip(sizes, ks):
            rem = t_j - k * CA
            if rem > 0:
                tot += math.ceil(rem / CB)
        return tot

    best = None

    def dfs(j, used_a, ks):
        nonlocal best
        if best is not None:
            return
        if j == n:
            if b_windows(ks) <= 8:
                best = list(ks)
            return
        kmax = min(8 - used_a, math.ceil(sizes[j] / CA))
        for k in range(kmax, -1, -1):
            ks.append(k)
            # quick prune: remaining jobs need B windows at least
            dfs(j + 1, used_a + k, ks)
            ks.pop()
            if best is not None:
                return

    dfs(0, 0, [])
    return best


def make_plan(W):
    """Choose template (CA, CB), per-core (region -> (expert, tokens))
    assignment, and host-computed leftover jobs.

    Returns (CA, CB, cores, host_jobs):
      cores[c] = {'A': (expert, token_idx) or None, 'B': ...}
      host_jobs = [(expert, token_idx)]
    expert == -1 means the shared expert (all tokens, combine weight 1).
    """
    sel = W > 0
    jobs = []
    for e in range(E):
        idx = np.nonzero(sel[:, e])[0]
        if len(idx):
            jobs.append((e, idx))
    jobs.append((-1, np.arange(T)))
    jobs.sort(key=lambda ei: -len(ei[1]))

    cands = []
    for CA in range(256, 513, 1):    # PSUM bank limit: cap <= 512
        for CB in range(32, CA + 1, 1):
            cost = max(2 * _WT_NS + (CA + CB) * _TOK_DMA_NS,
                       (CA + CB) * _TOK_PE_NS)
            cands.append((cost, CA, CB))
    cands.sort()

    best = None  # (cost, n_host, CA, CB, ks)
    for n_host in range(len(jobs)):
        dev = jobs[:len(jobs) - n_host]
        host = jobs[len(jobs) - n_host:]
        host_tok = sum(len(idx) for _, idx in host)
        if n_host and host_tok > 0.02 * (TOP_K * T + T):
            break  # refuse to push real work to the host
        sizes = [len(idx) for _, idx in dev]
        for cost, CA, CB in cands:
            if best is not None and cost >= best[0]:
                break  # cands sorted; nothing cheaper left for this n_host
            ks = _try_pack(sizes, CA, CB)
            if ks is not None:
                best = (cost, n_host, CA, CB, ks)
                break
    assert best is not None, "no feasible (CA, CB) template"
    _, n_host, CA, CB, ks = best
    dev = jobs[:len(jobs) - n_host]
    host = jobs[len(jobs) - n_host:]
    a_pieces, b_pieces = [], []
    for (e, idx), k in zip(dev, ks):
        pos = 0
        for i in range(k):
            take = min(CA, len(idx) - pos)
            if take <= 0:
                break
            a_pieces.append((e, idx[pos:pos + take]))
            pos += take
        while pos < len(idx):
            take = min(CB, len(idx) - pos)
            b_pieces.append((e, idx[pos:pos + take]))
            pos += take
    assert len(a_pieces) <= 8 and len(b_pieces) <= 8
    a_pieces += [None] * (8 - len(a_pieces))
    b_pieces += [None] * (8 - len(b_pieces))
    # pair large-A with small-B to even out the (ungraded) data
    a_pieces.sort(key=lambda p: -(len(p[1]) if p else 0))
    b_pieces.sort(key=lambda p: (len(p[1]) if p else 0))
    cores = [{'A': a_pieces[c], 'B': b_pieces[c]} for c in range(N_CORES)]
    return CA, CB, cores, host


def build_program(CA, CB):
    import concourse.bass as bass  # noqa: F401
    import concourse.mybir as mybir
    import concourse.tile as tile
    from concourse import bacc

    f32 = mybir.dt.float32
    bf16 = mybir.dt.bfloat16
    AF = mybir.ActivationFunctionType

    nc = bacc.Bacc()
    dram = {}
    for r, cap in (('a', CA), ('b', CB)):
        dram['wg' + r] = nc.dram_tensor('wg' + r, [P, KC, IC, P], bf16, kind="ExternalInput")
        dram['wu' + r] = nc.dram_tensor('wu' + r, [P, KC, IC, P], bf16, kind="ExternalInput")
        dram['wd' + r] = nc.dram_tensor('wd' + r, [P, IC, HC, P], bf16, kind="ExternalInput")
        dram['xt' + r] = nc.dram_tensor('xt' + r, [P, KC, cap], bf16, kind="ExternalInput")
        dram['y' + r] = nc.dram_tensor('y' + r, [P, HC, cap], bf16, kind="ExternalOutput")

    with tile.TileContext(nc) as tc:
        with tc.tile_pool(name="wts", bufs=1) as wpool, \
             tc.tile_pool(name="act", bufs=2) as apool, \
             tc.tile_pool(name="pp", bufs=4, space="PSUM") as pp:

            tiles = {}
            for r, cap in (('a', CA), ('b', CB)):
                tiles['wg' + r] = wpool.tile([P, KC, IC, P], bf16, name='WG' + r)
                tiles['wu' + r] = wpool.tile([P, KC, IC, P], bf16, name='WU' + r)
                tiles['wd' + r] = wpool.tile([P, IC, HC, P], bf16, name='WD' + r)
                tiles['xt' + r] = wpool.tile([P, KC, cap], bf16, name='XT' + r)
                tiles['y' + r] = wpool.tile([P, HC, cap], bf16, name='Y' + r)

            # DMA issue order: region A gate weights + x first (small leading
            # chunks so PE can start ~1us in), then up/down weights, then all
            # of region B.  Weights on the sync (SP) queue, x on scalar (Act)
            # so neither queue's sequencer becomes the bottleneck.
            def kc_chunks(nm, eng, groups):
                lo = 0
                for g in groups:
                    eng.dma_start(out=tiles[nm][:, lo:lo + g],
                                  in_=dram[nm][:, lo:lo + g])
                    lo += g

            kc_chunks('wga', nc.sync, (1, 1, 2, 4, 4, 4))
            kc_chunks('xta', nc.scalar, (1, 1, 2, 4, 4, 4))
            kc_chunks('wua', nc.sync, (4, 4, 4, 4))
            for g in range(4):  # wd chunked along hc (dim 2)
                nc.sync.dma_start(out=tiles['wda'][:, :, 4 * g:4 * g + 4, :],
                                  in_=dram['wda'][:, :, 4 * g:4 * g + 4, :])
            kc_chunks('wgb', nc.sync, (4, 4, 4, 4))
            kc_chunks('xtb', nc.scalar, (8, 8))
            kc_chunks('wub', nc.sync, (4, 4, 4, 4))
            for g in range(4):
                nc.sync.dma_start(out=tiles['wdb'][:, :, 4 * g:4 * g + 4, :],
                                  in_=dram['wdb'][:, :, 4 * g:4 * g + 4, :])

            # PE p-state warmup: dummy matmuls on a zeroed tile while the
            # first weight chunks stream in, so real matmuls run at full
            # clock.  Uses the 'pu' psum bufs (free until the up projection).
            import os
            n_warm = int(os.environ.get('KWARM', '10'))
            if n_warm:
                zt = apool.tile([P, 256], bf16, name='zwarm', tag='zw', bufs=1)
                nc.vector.memset(zt, 0.0)
                warm = pp.tile([P, 512], f32, name='pwarm', tag='pu', bufs=4)
                for i in range(n_warm):
                    nc.tensor.matmul(warm[:, :256], zt[:, :P], zt[:, :256],
                                     start=(i == 0), stop=(i == n_warm - 1))

            def slot(r, cap):
                WG, WU, WD = tiles['wg' + r], tiles['wu' + r], tiles['wd' + r]
                XT, Y = tiles['xt' + r], tiles['y' + r]
                # gate: kc-outer so matmuls consume weight/x chunks as they
                # arrive from HBM
                pg = [pp.tile([P, 512], f32, name=f'pg{r}{ic}', tag='pg', bufs=4)
                      for ic in range(IC)]
                for kc in range(KC):
                    for ic in range(IC):
                        nc.tensor.matmul(pg[ic][:, :cap], WG[:, kc, ic, :],
                                         XT[:, kc, :],
                                         start=(kc == 0), stop=(kc == KC - 1))
                # up: ic-outer so hh[ic] is ready early for the down phase
                hh = apool.tile([P, IC, cap], bf16, name=f'h{r}', tag='h', bufs=1)
                for ic in range(IC):
                    pu = pp.tile([P, 512], f32, name=f'pu{r}{ic}', tag='pu', bufs=4)
                    for kc in range(KC):
                        nc.tensor.matmul(pu[:, :cap], WU[:, kc, ic, :],
                                         XT[:, kc, :],
                                         start=(kc == 0), stop=(kc == KC - 1))
                    sl = apool.tile([P, 512], f32, name=f'sl{r}{ic}', tag='sl', bufs=2)
                    nc.scalar.activation(sl[:, :cap], pg[ic][:, :cap], AF.Sigmoid)
                    t1 = apool.tile([P, 512], f32, name=f't{r}{ic}', tag='t1', bufs=2)
                    nc.vector.tensor_mul(t1[:, :cap], sl[:, :cap], pg[ic][:, :cap])
                    nc.vector.tensor_mul(hh[:, ic, :], t1[:, :cap], pu[:, :cap])
                # down: stream y out every 4 hc chunks
                for hc in range(HC):
                    pd = pp.tile([P, 512], f32, name=f'pd{r}{hc}', tag='pg', bufs=4)
                    for ic in range(IC):
                        nc.tensor.matmul(pd[:, :cap], WD[:, ic, hc, :],
                                         hh[:, ic, :],
                                         start=(ic == 0), stop=(ic == IC - 1))
                    if hc % 2 == 0:
                        nc.scalar.activation(Y[:, hc, :], pd[:, :cap], AF.Copy)
                    else:
                        nc.vector.tensor_copy(out=Y[:, hc, :], in_=pd[:, :cap])
                # y store: bulk chunks on the sync queue; the last small chunk
                # on the scalar queue so it issues immediately after its copy
                # instead of queueing behind the bulk DMAs.
                for g0, gn in ((0, 4), (4, 4), (8, 4), (12, 2)):
                    nc.sync.dma_start(out=dram['y' + r][:, g0:g0 + gn, :],
                                      in_=Y[:, g0:g0 + gn, :])
                nc.scalar.dma_start(out=dram['y' + r][:, 14:16, :],
                                    in_=Y[:, 14:16, :])

            slot('a', CA)
            slot('b', CB)
    nc.finalize()
    return nc


def _pack_weight(wg, wu, wd, bf16):
    """-> (wg [P,KC,IC,P], wu same, wd [P,IC,HC,P]) in bf16."""
    wgp = np.ascontiguousarray(
        np.asarray(wg, np.float32).reshape(KC, P, IC, P).transpose(1, 0, 2, 3)
    ).astype(bf16)
    wup = np.ascontiguousarray(
        np.asarray(wu, np.float32).reshape(KC, P, IC, P).transpose(1, 0, 2, 3)
    ).astype(bf16)
    wdp = np.ascontiguousarray(
        np.asarray(wd, np.float32).reshape(IC, P, HC, P).transpose(1, 0, 2, 3)
    ).astype(bf16)
    return wgp, wup, wdp


def pack_inputs(CA, CB, cores, x, weights):
    import ml_dtypes
    bf16 = ml_dtypes.bfloat16
    w_gate, w_up, w_down, ws_gate, ws_up, ws_down = weights
    xT = np.ascontiguousarray(np.asarray(x, np.float32).T).astype(bf16)  # [H, T]

    wcache = {}

    def packed(e):
        if e not in wcache:
            if e == -1:
                wcache[e] = _pack_weight(ws_gate, ws_up, ws_down, bf16)
            else:
                wcache[e] = _pack_weight(w_gate[e], w_up[e], w_down[e], bf16)
        return wcache[e]

    zeros = (np.zeros((P, KC, IC, P), bf16),
             np.zeros((P, KC, IC, P), bf16),
             np.zeros((P, IC, HC, P), bf16))
    in_maps = []
    for c in range(N_CORES):
        m = {}
        for r, cap in (('a', CA), ('b', CB)):
            piece = cores[c]['A' if r == 'a' else 'B']
            if piece is None:
                wgp, wup, wdp = zeros
                xt = np.zeros((P, KC, cap), bf16)
            else:
                e, idx = piece
                wgp, wup, wdp = packed(e)
                xt = np.zeros((P, KC, cap), bf16)
                # xt[p, kc, c] = x[idx[c], kc*P + p]
                xt[:, :, :len(idx)] = xT[:, idx].reshape(KC, P, len(idx)).transpose(1, 0, 2)
            m['wg' + r], m['wu' + r], m['wd' + r] = wgp, wup, wdp
            m['xt' + r] = xt
        in_maps.append(m)
    return in_maps


def combine(CA, CB, cores, host_jobs, W, x, weights, results):
    w_gate, w_up, w_down, _, _, _ = weights
    out = np.zeros((T, H), np.float32)
    for c in range(N_CORES):
        for r, cap in (('a', CA), ('b', CB)):
            piece = cores[c]['A' if r == 'a' else 'B']
            if piece is None:
                continue
            e, idx = piece
            y = np.asarray(results[c]['y' + r], np.float32)  # [P, HC, cap]
            yf = y.transpose(2, 1, 0).reshape(cap, H)[:len(idx)]
            if e == -1:
                out[idx] += yf
            else:
                out[idx] += (ROUTED_SCALING * W[idx, e])[:, None] * yf
    xf = np.asarray(x, np.float32)
    for e, idx in host_jobs:
        if e == -1:
            wg, wu, wd = None, None, None
            g = xf[idx] @ np.asarray(weights[3], np.float32)
            u = xf[idx] @ np.asarray(weights[4], np.float32)
            h = g / (1.0 + np.exp(-g)) * u
            out[idx] += h @ np.asarray(weights[5], np.float32)
        else:
            g = xf[idx] @ np.asarray(w_gate[e], np.float32)
            u = xf[idx] @ np.asarray(w_up[e], np.float32)
            h = g / (1.0 + np.exp(-g)) * u
            y = h @ np.asarray(w_down[e], np.float32)
            out[idx] += (ROUTED_SCALING * W[idx, e])[:, None] * y
    return out


def prepare(**inputs):
    """Routing + planning + packing (everything except device execution)."""
    x = np.asarray(inputs["hidden_states"], np.float32)
    W = route_np(x, inputs["gate_w"], inputs["expert_bias"])
    CA, CB, cores, host_jobs = make_plan(W)
    weights = tuple(
        np.asarray(inputs[k], np.float32)
        for k in ("w_gate", "w_up", "w_down", "ws_gate", "ws_up", "ws_down"))
    in_maps = pack_inputs(CA, CB, cores, x, weights)
    return CA, CB, cores, host_jobs, W, weights, in_maps


def kernel(**inputs):
    from concourse.bass_utils import run_bass_kernel_spmd
    x = np.asarray(inputs["hidden_states"], np.float32)
    CA, CB, cores, host_jobs, W, weights, in_maps = prepare(**inputs)
    nc = build_program(CA, CB)
    res = run_bass_kernel_spmd(nc, in_maps, core_ids=list(range(N_CORES)))
    return combine(CA, CB, cores, host_jobs, W, x, weights, res.results)
